# revision 1
# baseline (speedup 1.0000x reference)
"""Dual-modality (opt/sar) multiplicative cross-attention — TRN2 Bass kernel.

Reference computation (per sample n, C=64, HW=64*64=4096):
  q_m = W_q^m x + b_q^m ; k_m = W_k^m x + b_k^m ; v_m = W_v^m x + b_v^m   (m in {opt,sar})
  att = softmax(q_o k_o^T) * softmax(q_s k_s^T)        (elementwise, (HW,HW))
  out = (att @ v_o) * (att @ v_s)                      -> (C,H,W) layout

Restructured for the hardware:
  A_m = exp(S_m - SHIFT)   (unnormalized, constant shift — safe: |S| <= ~62)
  sums_m[i] = sum_j A_m[i,j]  (free via ACT accum during the exp pass)
  P = A_o * A_s   ->  U_m = P @ v_m
  out = (U_o * U_s) / (sums_o * sums_s)^2     (denominators folded at the end)

Sharding: 8 cores, core c handles sample c//2, query-row half c%2 (2048 rows).
Each core gets the full sample x (for K/V) plus its row-half slice (for Q).

Dtypes: S matmuls in float32r (~1.7e-4, full PE rate at N=512);
A/P/v and the P@v matmuls in bf16 (validated 5.8e-3 max rel err vs fp32 ref);
sums/denominators in fp32.
"""
import numpy as np
from contextlib import ExitStack

import concourse.bass as bass
import concourse.tile as tile
from concourse import bacc, mybir
from concourse import masks
from concourse.bass_utils import run_bass_kernel_spmd

N, C, H, W = 4, 64, 64, 64
HW = H * W            # 4096
HALF = HW // 2        # 2048 query rows per core
NBLK = HALF // 128    # 16 q-blocks per core
SHIFT = 30.0

dt = mybir.dt
AF = mybir.ActivationFunctionType
ALU = mybir.AluOpType

_compiled = None


def _build(repeat=1):
    nc = bacc.Bacc("TRN2", debug=False)
    d_in = {}
    for m in ("opt", "sar"):
        d_in[f"x_{m}"] = nc.dram_tensor(f"x_{m}", (C, HW), dt.float32r, kind="ExternalInput").ap()
        d_in[f"xq_{m}"] = nc.dram_tensor(f"xq_{m}", (C, HALF), dt.float32r, kind="ExternalInput").ap()
        for p in ("q", "k", "v"):
            d_in[f"w{p}T_{m}"] = nc.dram_tensor(f"w{p}T_{m}", (C, C), dt.float32r, kind="ExternalInput").ap()
            d_in[f"b{p}_{m}"] = nc.dram_tensor(f"b{p}_{m}", (C, 1), dt.float32, kind="ExternalInput").ap()
    d_out = nc.dram_tensor("out", (C, HALF), dt.float32, kind="ExternalOutput").ap()

    with tile.TileContext(nc) as tc, ExitStack() as ctx:
        consts = ctx.enter_context(tc.tile_pool(name="consts", bufs=1))
        proj = ctx.enter_context(tc.tile_pool(name="proj", bufs=1))
        work = ctx.enter_context(tc.tile_pool(name="work", bufs=2))
        work1 = ctx.enter_context(tc.tile_pool(name="work1", bufs=2))
        stats = ctx.enter_context(tc.tile_pool(name="stats", bufs=2))
        outp = ctx.enter_context(tc.tile_pool(name="outp", bufs=1))
        ps_S = ctx.enter_context(tc.tile_pool(name="ps_S", bufs=2, space="PSUM"))
        ps_T = ctx.enter_context(tc.tile_pool(name="ps_T", bufs=2, space="PSUM"))
        ps_U = ctx.enter_context(tc.tile_pool(name="ps_U", bufs=1, space="PSUM"))
        ps_O = ctx.enter_context(tc.tile_pool(name="ps_O", bufs=1, space="PSUM"))

        ident_bf = consts.tile([128, 128], dt.bfloat16)
        masks.make_identity(nc, ident_bf[:])
        ident_f32 = consts.tile([128, 128], dt.float32)
        masks.make_identity(nc, ident_f32[:])
        neg_shift = consts.tile([128, 1], dt.float32)
        nc.gpsimd.memset(neg_shift[:], -SHIFT)
        warm = consts.tile([128, 1], dt.float32)
        nc.scalar.activation(warm[:], neg_shift[:], AF.Exp)

        # ---- load inputs: weights (tiny, ACT queue) then x (bulk, both queues) ----
        wr = {}
        bias = {}
        for m in ("opt", "sar"):
            for p in ("q", "k", "v"):
                wtr = consts.tile([C, C], dt.float32r, tag=f"wr_{p}_{m}")
                nc.scalar.dma_start(wtr[:], d_in[f"w{p}T_{m}"][:])
                wr[(p, m)] = wtr
                bt = consts.tile([C, 1], dt.float32, tag=f"b_{p}_{m}")
                nc.scalar.dma_start(bt[:], d_in[f"b{p}_{m}"][:])
                bias[(p, m)] = bt

        xr_ctx = ExitStack()
        xrpool = xr_ctx.enter_context(tc.tile_pool(name="xr", bufs=1))
        x_r = {}
        xq_r = {}
        for m in ("opt", "sar"):
            eng = nc.sync if m == "opt" else nc.scalar
            xqr = xrpool.tile([C, HALF], dt.float32r, tag=f"xqr_{m}")
            for dc in range(2):
                eng.dma_start(xqr[:, bass.ts(dc, HALF // 2)],
                              d_in[f"xq_{m}"][:, bass.ts(dc, HALF // 2)])
            xq_r[m] = xqr
            xr = xrpool.tile([C, HW], dt.float32r, tag=f"xr_{m}")
            for dc in range(4):
                eng.dma_start(xr[:, bass.ts(dc, HW // 4)],
                              d_in[f"x_{m}"][:, bass.ts(dc, HW // 4)])
            x_r[m] = xr

        # ---- projections ----
        # kT_stack: rows 0:64 = kT_opt, rows 64:128 = kT_sar (float32r)
        kT = proj.tile([128, HW], dt.float32r)
        qT = proj.tile([128, HALF], dt.float32r)
        def _proj(dst, p, m, mi, xsrc, jc):
            rows = slice(64 * mi, 64 * mi + 64)
            ps = ps_T.tile([C, 512], dt.float32, tag="ps_T")
            nc.tensor.matmul(ps[:], wr[(p, m)][:], xsrc[:, bass.ts(jc, 512)],
                             start=True, stop=True)
            nc.vector.tensor_scalar_add(dst[rows, bass.ts(jc, 512)], ps[:],
                                        bias[(p, m)][:])
        for jc in range(HALF // 512):
            for mi, m in enumerate(("opt", "sar")):
                _proj(qT, "q", m, mi, xq_r[m], jc)
        for jc in range(HW // 512):
            for mi, m in enumerate(("opt", "sar")):
                _proj(kT, "k", m, mi, x_r[m], jc)

        # v_both (bf16): col block j (128 wide) = [v_opt tile (64 c-cols) | v_sar tile].
        # Emitted AFTER block 0's S/exp so the first S matmuls aren't queued behind v work.
        v_both = proj.tile([128, HW], dt.bfloat16)

        def emit_v_both():
            vT = {}
            for m in ("opt", "sar"):
                vt = proj.tile([C, HW], dt.bfloat16, tag=f"vT_{m}")
                for jc in range(HW // 512):
                    ps = ps_T.tile([C, 512], dt.float32, tag="ps_T")
                    nc.tensor.matmul(ps[:], wr[("v", m)][:], x_r[m][:, bass.ts(jc, 512)],
                                     start=True, stop=True)
                    nc.vector.tensor_scalar_add(vt[:, bass.ts(jc, 512)], ps[:],
                                                bias[("v", m)][:])
                vT[m] = vt
            for g in range(HW // 512):   # 8 groups of 4 (opt,sar) tile pairs
                psv = ps_T.tile([128, 512], dt.bfloat16, tag="ps_T")
                for t in range(4):
                    j = g * 4 + t
                    nc.tensor.transpose(psv[:, bass.ts(t, 128)][:, 0:64],
                                        vT["opt"][:, bass.ts(j, 128)], ident_bf[0:64, 0:64])
                    nc.tensor.transpose(psv[:, bass.ts(t, 128)][:, 64:128],
                                        vT["sar"][:, bass.ts(j, 128)], ident_bf[0:64, 0:64])
                nc.vector.tensor_copy(v_both[:, bass.ts(g, 512)], psv[:])

        out_stage = outp.tile([C, HALF], dt.float32)

        # ---- main loop over q-blocks ----
        def emit_S_exp(i):
            A = {}
            parts = {}
            for mi, m in enumerate(("opt", "sar")):
                rows = slice(64 * mi, 64 * mi + 64)
                At = work.tile([128, HW], dt.bfloat16, tag=f"A_{m}")
                pt = stats.tile([128, HW // 1024], dt.float32, tag=f"part_{m}")
                for jc in range(HW // 1024):
                    ps = ps_S.tile([128, 1024], dt.float32, tag="ps_S")
                    nc.tensor.matmul(ps[:, 0:512], qT[rows, bass.ts(i, 128)],
                                     kT[rows, bass.ts(2 * jc, 512)],
                                     tile_position=(64 * mi, 0), start=True, stop=True)
                    nc.tensor.matmul(ps[:, 512:1024], qT[rows, bass.ts(i, 128)],
                                     kT[rows, bass.ts(2 * jc + 1, 512)],
                                     tile_position=(64 * mi, 0), start=True, stop=True)
                    nc.scalar.activation(At[:, bass.ts(jc, 1024)], ps[:], AF.Exp,
                                         bias=neg_shift[:], accum_out=pt[:, jc:jc + 1])
                A[m] = At
                parts[m] = pt
            return A, parts

        def emit_rest(i, A, parts):
            P = work1.tile([128, HW], dt.bfloat16, tag="P")
            for jc in range(HW // 1024):
                nc.vector.tensor_mul(P[:, bass.ts(jc, 1024)],
                                     A["opt"][:, bass.ts(jc, 1024)],
                                     A["sar"][:, bass.ts(jc, 1024)])

            PT = work1.tile([128, HW], dt.bfloat16, tag="PT")
            for g in range(HW // 1024):
                pst = ps_T.tile([128, 1024], dt.bfloat16, tag="ps_T")
                for t in range(8):
                    j = g * 8 + t
                    nc.tensor.transpose(pst[:, bass.ts(t, 128)],
                                        P[:, bass.ts(j, 128)], ident_bf[:])
                nc.vector.tensor_copy(PT[:, bass.ts(g, 1024)], pst[:])

            U = ps_U.tile([128, 128], dt.float32, tag="U")
            for j in range(HW // 128):
                nc.tensor.matmul(U[:], PT[:, bass.ts(j, 128)], v_both[:, bass.ts(j, 128)],
                                 start=(j == 0), stop=(j == HW // 128 - 1))

            sums_o = stats.tile([128, 1], dt.float32, tag="sums_o")
            sums_s = stats.tile([128, 1], dt.float32, tag="sums_s")
            nc.vector.reduce_sum(sums_o[:], parts["opt"][:], axis=mybir.AxisListType.X)
            nc.vector.reduce_sum(sums_s[:], parts["sar"][:], axis=mybir.AxisListType.X)
            denom = stats.tile([128, 1], dt.float32, tag="denom")
            nc.vector.tensor_mul(denom[:], sums_o[:], sums_s[:])
            inv = stats.tile([128, 1], dt.float32, tag="inv")
            nc.vector.reciprocal(inv[:], denom[:])
            inv2 = stats.tile([128, 1], dt.float32, tag="inv2")
            nc.vector.tensor_mul(inv2[:], inv[:], inv[:])

            t0 = stats.tile([128, 64], dt.float32, tag="t0")
            nc.vector.tensor_scalar_mul(t0[:], U[:, 0:64], inv2[:])
            ob = stats.tile([128, 64], dt.float32, tag="ob")
            nc.vector.tensor_mul(ob[:], t0[:], U[:, 64:128])
            pso = ps_O.tile([C, 128], dt.float32, tag="ps_O")
            nc.tensor.transpose(pso[:], ob[:], ident_f32[:])
            nc.vector.tensor_copy(out_stage[:, bass.ts(i, 128)], pso[:])
            nc.sync.dma_start(d_out[:, bass.ts(i, 128)], out_stage[:, bass.ts(i, 128)])

        A0, parts0 = emit_S_exp(0)
        emit_v_both()
        xr_ctx.close()
        emit_rest(0, A0, parts0)
        for i in range(1, NBLK):
            A, parts = emit_S_exp(i)
            emit_rest(i, A, parts)
        for _r in range(repeat - 1):
            for i in range(NBLK):
                A, parts = emit_S_exp(i)
                emit_rest(i, A, parts)

    nc.compile()
    return nc


def _to_f32r(x):
    """Round fp32 to the float32r format: RNE to 11 mantissa bits, low 12 bits zero."""
    u = np.ascontiguousarray(x, np.float32).view(np.uint32)
    lsb = (u >> 12) & 1
    r = (u + np.uint32(0x7FF) + lsb) & np.uint32(0xFFFFF000)
    return r.view(np.float32)


def kernel(x_opt, x_sar, wq_opt, bq_opt, wk_opt, bk_opt, wv_opt, bv_opt,
           wq_sar, bq_sar, wk_sar, bk_sar, wv_sar, bv_sar, _trace=False):
    global _compiled
    if _compiled is None:
        _compiled = _build()
    nc = _compiled

    ws = {"wqT_opt": wq_opt.T, "wkT_opt": wk_opt.T, "wvT_opt": wv_opt.T,
          "wqT_sar": wq_sar.T, "wkT_sar": wk_sar.T, "wvT_sar": wv_sar.T}
    bs = {"bq_opt": bq_opt, "bk_opt": bk_opt, "bv_opt": bv_opt,
          "bq_sar": bq_sar, "bk_sar": bk_sar, "bv_sar": bv_sar}
    ws = {k: _to_f32r(v) for k, v in ws.items()}
    bs = {k: np.ascontiguousarray(np.asarray(v, np.float32).reshape(C, 1)) for k, v in bs.items()}

    in_maps = []
    for core in range(8):
        n, h = core // 2, core % 2
        m = dict(ws)
        m.update(bs)
        xo = _to_f32r(np.asarray(x_opt[n], np.float32).reshape(C, HW))
        xs = _to_f32r(np.asarray(x_sar[n], np.float32).reshape(C, HW))
        m["x_opt"] = xo
        m["x_sar"] = xs
        m["xq_opt"] = np.ascontiguousarray(xo[:, h * HALF:(h + 1) * HALF])
        m["xq_sar"] = np.ascontiguousarray(xs[:, h * HALF:(h + 1) * HALF])
        in_maps.append(m)

    r = run_bass_kernel_spmd(nc, in_maps, core_ids=list(range(8)), trace=_trace)
    out = np.empty((N, C, HW), np.float32)
    for core in range(8):
        n, h = core // 2, core % 2
        out[n][:, h * HALF:(h + 1) * HALF] = r.results[core]["out"]
    kernel._last_result = r
    return out.reshape(N, C, H, W)

